# revision 1
# baseline (speedup 1.0000x reference)
"""KGNN head kernel for Trainium2 (Bass/Tile), 8-core data-parallel SPMD.

Computation (per batch b):
    score[g, n] = sum_d drug[b, g, d] * rel[b, 8g+n, d]         (n in 0..8)
    out[b, g, d] = sum_n score[g, n] * ent[b, 8g+n, d]

Layout: one SBUF partition holds one (batch-pair-slot, group) row; the 8
neighbors x 64 dims of that group lie contiguously in the free dimension, so
every DMA is a straight contiguous stream (2KB runs per partition).

Per 64-element chunk:
  - DVE tensor_tensor_reduce fuses the (rel * drug) multiply with the dot
    over d -> per-partition score scalar.
  - tensor_scalar (DVE) / activation-with-scale (ACT) scales the matching
    ent chunk by the score.
  - The sum over the 8 neighbor chunks runs on the idle TensorEngine as
    PSUM-accumulating matmuls with a constant 128x128 identity as lhsT.
"""

import numpy as np

import concourse.bass as bass  # noqa: F401  (engine namespaces via nc)
import concourse.mybir as mybir
import concourse.tile as tile
from concourse import bacc
from concourse.bass_utils import run_bass_kernel_spmd
from concourse.masks import make_identity

F32 = mybir.dt.float32

N_CORES = 8
B_FULL = 2048
B_LOCAL = B_FULL // N_CORES  # 256
G = 64          # groups per sample
NN = 8          # neighbors per group
D = 64          # feature dim
S = G * NN      # 512 neighbor slots

SB = 16                 # batches per superblock
U = SB // 2             # 2-batch units per superblock (8)
N_SBLK = B_LOCAL // SB  # superblocks per core (16)

# chunks (of 8) whose ent-scaling runs on DVE; the rest go to ACT
DVE_W_CHUNKS = 2


def _build_nc(b_local: int = B_LOCAL) -> "bacc.Bacc":
    n_sblk = b_local // SB
    assert n_sblk * SB == b_local

    nc = bacc.Bacc("TRN2", target_bir_lowering=False, debug=False)

    drug_d = nc.dram_tensor("drug", [b_local, G, D], F32, kind="ExternalInput")
    rel_d = nc.dram_tensor("rel", [b_local, S, D], F32, kind="ExternalInput")
    ent_d = nc.dram_tensor("ent", [b_local, S, D], F32, kind="ExternalInput")
    out_d = nc.dram_tensor("out", [b_local, G, D], F32, kind="ExternalOutput")

    # partition p = (bb g); free = [u][(n d)] / [u][d]
    rel_v = rel_d[:].rearrange(
        "(s u bb) (g n) d -> s (bb g) u (n d)", s=n_sblk, u=U, bb=2, g=G, n=NN
    )
    ent_v = ent_d[:].rearrange(
        "(s u bb) (g n) d -> s (bb g) u (n d)", s=n_sblk, u=U, bb=2, g=G, n=NN
    )
    drug_v = drug_d[:].rearrange(
        "(s u bb) g d -> s (bb g) u d", s=n_sblk, u=U, bb=2
    )
    out_v = out_d[:].rearrange(
        "(s u bb) g d -> s (bb g) u d", s=n_sblk, u=U, bb=2
    )

    with tile.TileContext(nc) as tc:
        with (
            tc.tile_pool(name="const", bufs=1) as const_pool,
            tc.tile_pool(name="rel", bufs=2) as rel_pool,
            tc.tile_pool(name="ent", bufs=2) as ent_pool,
            tc.tile_pool(name="drug", bufs=2) as drug_pool,
            tc.tile_pool(name="score", bufs=3) as score_pool,
            tc.tile_pool(name="prod", bufs=2) as prod_pool,
            tc.tile_pool(name="w", bufs=3) as w_pool,
            tc.tile_pool(name="outs", bufs=2) as out_pool,
            tc.tile_pool(name="psum", bufs=2, space="PSUM") as psum_pool,
        ):
            ident = const_pool.tile([128, 128], F32)
            make_identity(nc, ident[:])

            for sb in range(n_sblk):
                rel_t = rel_pool.tile([128, U * NN * D], F32)
                nc.sync.dma_start(
                    out=rel_t[:].rearrange("p (u nd) -> p u nd", u=U), in_=rel_v[sb]
                )
                ent_t = ent_pool.tile([128, U * NN * D], F32)
                nc.sync.dma_start(
                    out=ent_t[:].rearrange("p (u nd) -> p u nd", u=U), in_=ent_v[sb]
                )
                drug_t = drug_pool.tile([128, U * D], F32)
                nc.sync.dma_start(
                    out=drug_t[:].rearrange("p (u d) -> p u d", u=U), in_=drug_v[sb]
                )

                # scores: one broadcast multiply + one segmented reduce
                # (tensor_tensor_reduce is broken on the HW path, probed)
                prod_t = prod_pool.tile([128, U * NN * D], F32)
                nc.vector.tensor_tensor(
                    out=prod_t[:].rearrange("p (u n d) -> p u n d", u=U, n=NN),
                    in0=rel_t[:].rearrange("p (u n d) -> p u n d", u=U, n=NN),
                    in1=drug_t[:]
                    .rearrange("p (u n d) -> p u n d", u=U, n=1)
                    .to_broadcast([128, U, NN, D]),
                    op=mybir.AluOpType.mult,
                )
                score_t = score_pool.tile([128, U * NN], F32)
                nc.vector.tensor_reduce(
                    out=score_t[:],
                    in_=prod_t[:].rearrange("p (un d) -> p un d", d=D),
                    axis=mybir.AxisListType.X,
                    op=mybir.AluOpType.add,
                )

                # weighted ent chunks, accumulated over n on the TensorEngine
                psum_t = psum_pool.tile([128, U * D], F32)
                for c in range(NN):
                    w_t = w_pool.tile([128, U * D], F32)
                    for u in range(U):
                        off = u * NN * D + c * D
                        src = ent_t[:, off : off + D]
                        dst = w_t[:, u * D : (u + 1) * D]
                        sc_ap = score_t[:, u * NN + c : u * NN + c + 1]
                        if c < DVE_W_CHUNKS:
                            nc.vector.tensor_scalar_mul(dst, src, sc_ap)
                        else:
                            nc.scalar.mul(dst, src, sc_ap)
                    nc.tensor.matmul(
                        out=psum_t[:],
                        lhsT=ident[:],
                        rhs=w_t[:],
                        start=(c == 0),
                        stop=(c == NN - 1),
                    )

                out_t = out_pool.tile([128, U * D], F32)
                nc.scalar.copy(out=out_t[:], in_=psum_t[:])
                nc.sync.dma_start(
                    out=out_v[sb],
                    in_=out_t[:].rearrange("p (u d) -> p u d", u=U),
                )

    nc.compile()
    return nc


_NC_CACHE: dict = {}


def _get_nc(b_local: int = B_LOCAL):
    if b_local not in _NC_CACHE:
        _NC_CACHE[b_local] = _build_nc(b_local)
    return _NC_CACHE[b_local]


def run_sharded(drug, rel, ent, trace: bool = False):
    """Shard batch dim across the 8 cores, run, gather. Returns
    (full output [B, G, D], BassKernelResults)."""
    drug = np.ascontiguousarray(np.asarray(drug, dtype=np.float32))
    rel = np.ascontiguousarray(np.asarray(rel, dtype=np.float32))
    ent = np.ascontiguousarray(np.asarray(ent, dtype=np.float32))
    b = drug.shape[0]
    nb = b // N_CORES
    assert nb * N_CORES == b
    nc = _get_nc(nb)
    in_maps = [
        {
            "drug": np.ascontiguousarray(drug[i * nb : (i + 1) * nb]),
            "rel": np.ascontiguousarray(rel[i * nb : (i + 1) * nb]),
            "ent": np.ascontiguousarray(ent[i * nb : (i + 1) * nb]),
        }
        for i in range(N_CORES)
    ]
    last_exc = None
    for attempt in range(3):
        try:
            res = run_bass_kernel_spmd(nc, in_maps, list(range(N_CORES)), trace=trace)
            break
        except Exception as exc:  # transient device-unrecoverable states
            last_exc = exc
            import time

            time.sleep(10 * (attempt + 1))
    else:
        raise last_exc
    out = np.concatenate([res.results[i]["out"] for i in range(N_CORES)], axis=0)
    return out, res


def kernel(drug, rel, ent):
    out, _ = run_sharded(drug, rel, ent, trace=False)
    return out



# revision 2
# speedup vs baseline: 1.1788x; 1.1788x over previous
"""KGNN head kernel for Trainium2 (Bass/Tile), 8-core data-parallel SPMD.

Computation (per batch b):
    score[g, n] = sum_d drug[b, g, d] * rel[b, 8g+n, d]         (n in 0..8)
    out[b, g, d] = sum_n score[g, n] * ent[b, 8g+n, d]

Layout: one SBUF partition holds one (batch-pair-slot, group) row; the 8
neighbors x 64 dims of that group lie contiguously in the free dimension, so
every DMA is a straight contiguous stream (2KB runs per partition).

Per superblock (16 batches):
  - DVE tensor_tensor (rel * drug broadcast over n) then tensor_reduce over
    d -> score [128, U*NN] fp32.
  - GpSimd tensor_tensor scales ent by score (score broadcast over d) in ONE
    instruction, writing w in bf16 with the neighbor axis outermost.
  - The sum over the 8 neighbor chunks runs on the TensorEngine as
    PSUM-accumulating bf16 matmuls with a constant bf16 identity as lhsT.
  - ACT copies PSUM -> SBUF for the output DMA.
"""

import numpy as np

import concourse.bass as bass  # noqa: F401  (engine namespaces via nc)
import concourse.mybir as mybir
import concourse.tile as tile
from concourse import bacc
from concourse.bass_utils import run_bass_kernel_spmd
from concourse.masks import make_identity

F32 = mybir.dt.float32
BF16 = mybir.dt.bfloat16

N_CORES = 8
B_FULL = 2048
B_LOCAL = B_FULL // N_CORES  # 256
G = 64          # groups per sample
NN = 8          # neighbors per group
D = 64          # feature dim
S = G * NN      # 512 neighbor slots

SB = 16                 # batches per superblock
U = SB // 2             # 2-batch units per superblock (8)
N_SBLK = B_LOCAL // SB  # superblocks per core (16)

# engine for the ent*score scale: "gpsimd" keeps DVE free for score work
WSCALE_ENGINE = "gpsimd"


def _build_nc(b_local: int = B_LOCAL) -> "bacc.Bacc":
    n_sblk = b_local // SB
    assert n_sblk * SB == b_local

    nc = bacc.Bacc("TRN2", target_bir_lowering=False, debug=False)

    drug_d = nc.dram_tensor("drug", [b_local, G, D], F32, kind="ExternalInput")
    rel_d = nc.dram_tensor("rel", [b_local, S, D], F32, kind="ExternalInput")
    ent_d = nc.dram_tensor("ent", [b_local, S, D], F32, kind="ExternalInput")
    out_d = nc.dram_tensor("out", [b_local, G, D], F32, kind="ExternalOutput")

    # partition p = (bb g); free = [u][(n d)] / [u][d]
    rel_v = rel_d[:].rearrange(
        "(s u bb) (g n) d -> s (bb g) u (n d)", s=n_sblk, u=U, bb=2, g=G, n=NN
    )
    ent_v = ent_d[:].rearrange(
        "(s u bb) (g n) d -> s (bb g) u (n d)", s=n_sblk, u=U, bb=2, g=G, n=NN
    )
    drug_v = drug_d[:].rearrange(
        "(s u bb) g d -> s (bb g) u d", s=n_sblk, u=U, bb=2
    )
    out_v = out_d[:].rearrange(
        "(s u bb) g d -> s (bb g) u d", s=n_sblk, u=U, bb=2
    )

    with tile.TileContext(nc) as tc:
        with (
            tc.tile_pool(name="const", bufs=1) as const_pool,
            tc.tile_pool(name="rel", bufs=3) as rel_pool,
            tc.tile_pool(name="ent", bufs=3) as ent_pool,
            tc.tile_pool(name="drug", bufs=3) as drug_pool,
            tc.tile_pool(name="score", bufs=3) as score_pool,
            tc.tile_pool(name="prod", bufs=2) as prod_pool,
            tc.tile_pool(name="w", bufs=2) as w_pool,
            tc.tile_pool(name="outs", bufs=2) as out_pool,
            tc.tile_pool(name="psum", bufs=2, space="PSUM") as psum_pool,
        ):
            ident = const_pool.tile([128, 128], BF16)
            make_identity(nc, ident[:])

            for sb in range(n_sblk):
                rel_t = rel_pool.tile([128, U * NN * D], F32)
                nc.sync.dma_start(
                    out=rel_t[:].rearrange("p (u nd) -> p u nd", u=U), in_=rel_v[sb]
                )
                ent_t = ent_pool.tile([128, U * NN * D], F32)
                nc.sync.dma_start(
                    out=ent_t[:].rearrange("p (u nd) -> p u nd", u=U), in_=ent_v[sb]
                )
                drug_t = drug_pool.tile([128, U * D], F32)
                nc.sync.dma_start(
                    out=drug_t[:].rearrange("p (u d) -> p u d", u=U), in_=drug_v[sb]
                )

                # scores: one broadcast multiply + one segmented reduce
                # (tensor_tensor_reduce is broken on the HW path, probed)
                prod_t = prod_pool.tile([128, U * NN * D], F32)
                nc.vector.tensor_tensor(
                    out=prod_t[:].rearrange("p (u n d) -> p u n d", u=U, n=NN),
                    in0=rel_t[:].rearrange("p (u n d) -> p u n d", u=U, n=NN),
                    in1=drug_t[:]
                    .rearrange("p (u n d) -> p u n d", u=U, n=1)
                    .to_broadcast([128, U, NN, D]),
                    op=mybir.AluOpType.mult,
                )
                score_t = score_pool.tile([128, U * NN], F32)
                nc.vector.tensor_reduce(
                    out=score_t[:],
                    in_=prod_t[:].rearrange("p (un d) -> p un d", d=D),
                    axis=mybir.AxisListType.X,
                    op=mybir.AluOpType.add,
                )

                # w[n, u, d] = score[u, n] * ent[u, n, d] in ONE instruction,
                # bf16 out, neighbor axis outermost so each matmul chunk is
                # contiguous
                w_t = w_pool.tile([128, NN * U * D], BF16)
                weng = nc.gpsimd if WSCALE_ENGINE == "gpsimd" else nc.vector
                weng.tensor_tensor(
                    out=w_t[:].rearrange("p (n u d) -> p u n d", n=NN, u=U),
                    in0=ent_t[:].rearrange("p (u n d) -> p u n d", u=U, n=NN),
                    in1=score_t[:]
                    .rearrange("p (u n) -> p u n", u=U)
                    .unsqueeze(3)
                    .to_broadcast([128, U, NN, D]),
                    op=mybir.AluOpType.mult,
                )

                # sum over n: 8 PSUM-accumulating identity matmuls (bf16)
                psum_t = psum_pool.tile([128, U * D], F32)
                for c in range(NN):
                    nc.tensor.matmul(
                        out=psum_t[:],
                        lhsT=ident[:],
                        rhs=w_t[:, c * U * D : (c + 1) * U * D],
                        start=(c == 0),
                        stop=(c == NN - 1),
                    )

                out_t = out_pool.tile([128, U * D], F32)
                nc.scalar.copy(out=out_t[:], in_=psum_t[:])
                nc.sync.dma_start(
                    out=out_v[sb],
                    in_=out_t[:].rearrange("p (u d) -> p u d", u=U),
                )

    nc.compile()
    return nc


_NC_CACHE: dict = {}


def _get_nc(b_local: int = B_LOCAL):
    if b_local not in _NC_CACHE:
        _NC_CACHE[b_local] = _build_nc(b_local)
    return _NC_CACHE[b_local]


def run_sharded(drug, rel, ent, trace: bool = False):
    """Shard batch dim across the 8 cores, run, gather. Returns
    (full output [B, G, D], BassKernelResults)."""
    drug = np.ascontiguousarray(np.asarray(drug, dtype=np.float32))
    rel = np.ascontiguousarray(np.asarray(rel, dtype=np.float32))
    ent = np.ascontiguousarray(np.asarray(ent, dtype=np.float32))
    b = drug.shape[0]
    nb = b // N_CORES
    assert nb * N_CORES == b
    nc = _get_nc(nb)
    in_maps = [
        {
            "drug": np.ascontiguousarray(drug[i * nb : (i + 1) * nb]),
            "rel": np.ascontiguousarray(rel[i * nb : (i + 1) * nb]),
            "ent": np.ascontiguousarray(ent[i * nb : (i + 1) * nb]),
        }
        for i in range(N_CORES)
    ]
    last_exc = None
    for attempt in range(3):
        try:
            res = run_bass_kernel_spmd(nc, in_maps, list(range(N_CORES)), trace=trace)
            break
        except Exception as exc:  # transient device-unrecoverable states
            last_exc = exc
            import time

            time.sleep(10 * (attempt + 1))
    else:
        raise last_exc
    out = np.concatenate([res.results[i]["out"] for i in range(N_CORES)], axis=0)
    return out, res


def kernel(drug, rel, ent):
    out, _ = run_sharded(drug, rel, ent, trace=False)
    return out


# revision 10
# speedup vs baseline: 1.5587x; 1.3223x over previous
"""KGNN head kernel for Trainium2 (Bass/Tile), 8-core data-parallel SPMD.

Computation (per batch b):
    score[g, n] = sum_d drug[b, g, d] * rel[b, 8g+n, d]         (n in 0..8)
    out[b, g, d] = sum_n score[g, n] * ent[b, 8g+n, d]

Layout: one SBUF partition holds one BATCH (two 128-batch blocks per core);
a tile covers 8 groups x 8 neighbors x 64 dims = 4096 elements of free dim.
Per-partition HBM runs are 16KiB for rel/ent and 2KiB for drug/out, so DMA
descriptors are large and sequential.

Per tile (pb, gt):
  - DVE tensor_tensor (rel * drug broadcast over n) -> prod bf16.
  - d-reduction: two bf16 2x fold-adds (64->32->16) + one tensor_reduce
    (16->1) -> score fp32 [128, gc*NN].
  - DVE tensor_tensor scales ent by score (broadcast over d), writing w in
    bf16 with the neighbor axis outermost.
  - Sum over the 8 neighbor chunks: PSUM-accumulating bf16 matmuls with a
    constant bf16 identity as lhsT.
  - ACT copies PSUM -> SBUF and issues the output DMA from its own HWDGE
    queue (keeps the SP queue free for input loads).
"""

import numpy as np

import concourse.bass as bass  # noqa: F401  (engine namespaces via nc)
import concourse.mybir as mybir
import concourse.tile as tile
from concourse import bacc
from concourse.bass_utils import run_bass_kernel_spmd
from concourse.masks import make_identity

F32 = mybir.dt.float32
BF16 = mybir.dt.bfloat16

N_CORES = 8
B_FULL = 2048
B_LOCAL = B_FULL // N_CORES  # 256
G = 64          # groups per sample
NN = 8          # neighbors per group
D = 64          # feature dim
S = G * NN      # 512 neighbor slots

PB = B_LOCAL // 128  # 128-batch partition blocks (2)
GT = 8               # group tiles
GC = G // GT         # groups per tile (8)
FD = GC * NN * D     # free dim per tile (4096)


def _build_nc(b_local: int = B_LOCAL) -> "bacc.Bacc":
    pb_n = b_local // 128
    assert pb_n * 128 == b_local

    nc = bacc.Bacc("TRN2", target_bir_lowering=False, debug=False)

    drug_d = nc.dram_tensor("drug", [b_local, G, D], F32, kind="ExternalInput")
    rel_d = nc.dram_tensor("rel", [b_local, S, D], F32, kind="ExternalInput")
    ent_d = nc.dram_tensor("ent", [b_local, S, D], F32, kind="ExternalInput")
    out_d = nc.dram_tensor("out", [b_local, G, D], F32, kind="ExternalOutput")

    rel_v = rel_d[:].rearrange(
        "(pb p) (gt gc n) d -> pb gt p (gc n d)", pb=pb_n, gt=GT, gc=GC, n=NN
    )
    ent_v = ent_d[:].rearrange(
        "(pb p) (gt gc n) d -> pb gt p (gc n d)", pb=pb_n, gt=GT, gc=GC, n=NN
    )
    drug_v = drug_d[:].rearrange("(pb p) g d -> pb p (g d)", pb=pb_n)
    out_v = out_d[:].rearrange(
        "(pb p) (gt gc) d -> pb gt p (gc d)", pb=pb_n, gt=GT, gc=GC
    )

    with tile.TileContext(nc) as tc:
        with (
            tc.tile_pool(name="const", bufs=1) as const_pool,
            tc.tile_pool(name="rel", bufs=3) as rel_pool,
            tc.tile_pool(name="ent", bufs=3) as ent_pool,
            tc.tile_pool(name="drug", bufs=1) as drug_pool,
            tc.tile_pool(name="score", bufs=3) as score_pool,
            tc.tile_pool(name="prod", bufs=2) as prod_pool,
            tc.tile_pool(name="fold", bufs=2) as fold_pool,
            tc.tile_pool(name="w", bufs=2) as w_pool,
            tc.tile_pool(name="outs", bufs=2) as out_pool,
            tc.tile_pool(name="psum", bufs=2, space="PSUM") as psum_pool,
        ):
            ident = const_pool.tile([128, 128], BF16)
            make_identity(nc, ident[:])

            # preload all drug rows (16KiB/partition per block, fp32)
            drug_t = drug_pool.tile([128, pb_n * G * D], F32)
            drug_view = drug_t[:].rearrange(
                "p (pb g d) -> p pb g d", pb=pb_n, g=G
            )
            for pb in range(pb_n):
                nc.sync.dma_start(
                    out=drug_t[:, pb * G * D : (pb + 1) * G * D], in_=drug_v[pb]
                )

            for t in range(pb_n * GT):
                pb, gt = divmod(t, GT)
                rel_t = rel_pool.tile([128, FD], F32)
                nc.sync.dma_start(out=rel_t[:], in_=rel_v[pb, gt])
                ent_t = ent_pool.tile([128, FD], F32)
                nc.sync.dma_start(out=ent_t[:], in_=ent_v[pb, gt])

                # prod = rel * drug (broadcast over n), bf16 out
                prod_t = prod_pool.tile([128, FD], BF16)
                nc.vector.tensor_tensor(
                    out=prod_t[:].rearrange("p (gc n d) -> p gc n d", gc=GC, n=NN),
                    in0=rel_t[:].rearrange("p (gc n d) -> p gc n d", gc=GC, n=NN),
                    in1=drug_view[:, pb, gt * GC : (gt + 1) * GC]
                    .unsqueeze(2)
                    .to_broadcast([128, GC, NN, D]),
                    op=mybir.AluOpType.mult,
                )

                # d-reduction: bf16 2x folds 64->32->16, then reduce 16->1
                un = GC * NN  # 64 segments
                f1_t = fold_pool.tile([128, un * (D // 2) + un * (D // 4)], BF16)
                f1 = f1_t[:, : un * (D // 2)].rearrange(
                    "p (un h) -> p un h", un=un
                )
                f2 = f1_t[:, un * (D // 2) :].rearrange(
                    "p (un q) -> p un q", un=un
                )
                pv = prod_t[:].rearrange("p (un d) -> p un d", un=un)
                nc.vector.tensor_tensor(
                    out=f1, in0=pv[:, :, 0 : D // 2], in1=pv[:, :, D // 2 : D],
                    op=mybir.AluOpType.add,
                )
                nc.vector.tensor_tensor(
                    out=f2, in0=f1[:, :, 0 : D // 4], in1=f1[:, :, D // 4 : D // 2],
                    op=mybir.AluOpType.add,
                )
                score_t = score_pool.tile([128, un], F32)
                nc.vector.tensor_reduce(
                    out=score_t[:],
                    in_=f2,
                    axis=mybir.AxisListType.X,
                    op=mybir.AluOpType.add,
                )

                # w[n, gc, d] = score[gc, n] * ent[gc, n, d], bf16 out
                w_t = w_pool.tile([128, FD], BF16)
                nc.vector.tensor_tensor(
                    out=w_t[:].rearrange("p (n gc d) -> p gc n d", n=NN, gc=GC),
                    in0=ent_t[:].rearrange("p (gc n d) -> p gc n d", gc=GC, n=NN),
                    in1=score_t[:]
                    .rearrange("p (gc n) -> p gc n", gc=GC)
                    .unsqueeze(3)
                    .to_broadcast([128, GC, NN, D]),
                    op=mybir.AluOpType.mult,
                )

                # sum over n: 8 PSUM-accumulating identity matmuls (bf16)
                psum_t = psum_pool.tile([128, GC * D], F32)
                for c in range(NN):
                    nc.tensor.matmul(
                        out=psum_t[:],
                        lhsT=ident[:],
                        rhs=w_t[:, c * GC * D : (c + 1) * GC * D],
                        start=(c == 0),
                        stop=(c == NN - 1),
                    )

                out_t = out_pool.tile([128, GC * D], F32)
                nc.scalar.copy(out=out_t[:], in_=psum_t[:])
                nc.scalar.dma_start(out=out_v[pb, gt], in_=out_t[:])

    nc.compile()
    return nc


_NC_CACHE: dict = {}


def _get_nc(b_local: int = B_LOCAL):
    if b_local not in _NC_CACHE:
        _NC_CACHE[b_local] = _build_nc(b_local)
    return _NC_CACHE[b_local]


def run_sharded(drug, rel, ent, trace: bool = False):
    """Shard batch dim across the 8 cores, run, gather. Returns
    (full output [B, G, D], BassKernelResults)."""
    drug = np.ascontiguousarray(np.asarray(drug, dtype=np.float32))
    rel = np.ascontiguousarray(np.asarray(rel, dtype=np.float32))
    ent = np.ascontiguousarray(np.asarray(ent, dtype=np.float32))
    b = drug.shape[0]
    nb = b // N_CORES
    assert nb * N_CORES == b
    nc = _get_nc(nb)
    in_maps = [
        {
            "drug": np.ascontiguousarray(drug[i * nb : (i + 1) * nb]),
            "rel": np.ascontiguousarray(rel[i * nb : (i + 1) * nb]),
            "ent": np.ascontiguousarray(ent[i * nb : (i + 1) * nb]),
        }
        for i in range(N_CORES)
    ]
    last_exc = None
    for attempt in range(3):
        try:
            res = run_bass_kernel_spmd(nc, in_maps, list(range(N_CORES)), trace=trace)
            break
        except Exception as exc:  # transient device-unrecoverable states
            last_exc = exc
            import time

            time.sleep(10 * (attempt + 1))
    else:
        raise last_exc
    out = np.concatenate([res.results[i]["out"] for i in range(N_CORES)], axis=0)
    return out, res


def kernel(drug, rel, ent):
    out, _ = run_sharded(drug, rel, ent, trace=False)
    return out
